# revision 32
# baseline (speedup 1.0000x reference)
"""KeOps-style multi-head attention (unnormalized-exp softmax) on 8 trn2 cores.

Sharding: core c handles batch bi = c//2 and query rows u*1024..(u+1)*1024
(u = c%2), ALL 8 heads. Output is a pure concat over cores (no reduction).

Engine budget per core (under the sustained-load PE clock throttle to
~1.2 GHz): the ACT engine's 128 exp instructions ([128,1024] psum->sbuf,
~1.0-1.2us each) form the ~140us spine; everything else hides under it.
All matmuls are bf16 (1 col/cycle on the PE, FWL-eligible weight loads;
fp32 matmuls are 4x slower and poison FWL for the following matmul), with
fp32 PSUM accumulation. The host pre-casts x and weights to bf16, which
also halves input DMA bytes.

Structure per (g=512 queries, hh=head-tensor, pp=head-pair) block:
scores^T chunks [128 keys, 2x512 queries] via row-tiled K=32 bf16
matmuls (the two heads' matmuls overlap on different PE row strips) ->
exp on ACT -> numer^T accumulation matmuls with e^T as the moving
operand and [v | ones] as the stationary (the ones column yields the
softmax denominator for free).

Program order matters because engines execute in-order: the attention
blocks are emitted as soon as kT[0]/v/qT[0] exist; the remaining
projections, the per-(g,hh) denom->recip->broadcast->normalize chains,
and the output projection are placed between later blocks so they
overlap the ACT-bound loop. x arrives pre-transposed via DMA-transpose
loads (no PE transposes). PSUM: 3x2 banks scores + 2 banks numer.
"""

import numpy as np
import ml_dtypes
from contextlib import ExitStack

import concourse.bass as bass
import concourse.mybir as mybir
import concourse.tile as tile
from concourse import bacc
from concourse.bass_utils import run_bass_kernel_spmd

DIM = 256
NUM_HEADS = 8
HEAD_DIM = 32
B = 4
N = 2048
NQ = 1024          # query rows per core
NCORES = 8
FP = mybir.dt.float32
BF = mybir.dt.bfloat16
EXP = mybir.ActivationFunctionType.Exp

NT_KV = N // 128   # 16 n-tiles of kv rows
NGQ = NQ // 512    # 2 groups of 512 query cols in q^T
NGK = N // 512     # 4 groups in k^T free dim
NJ = N // 128      # 16 key chunks of 128
NT_Q = NQ // 128   # 8 output row tiles


def build_program():
    nc = bacc.Bacc()

    xq = nc.declare_dram_parameter("xq", [NQ, DIM], BF, isOutput=False)
    xkv = nc.declare_dram_parameter("xkv", [N, DIM], BF, isOutput=False)
    wq = nc.declare_dram_parameter("wq", [DIM, DIM], BF, isOutput=False)
    wk = nc.declare_dram_parameter("wk", [DIM, DIM], BF, isOutput=False)
    wv = nc.declare_dram_parameter("wv", [DIM, DIM], BF, isOutput=False)
    wout = nc.declare_dram_parameter("wout", [DIM, DIM], BF, isOutput=False)
    bout = nc.declare_dram_parameter("bout", [DIM], FP, isOutput=False)
    out = nc.declare_dram_parameter("out", [NQ, DIM], FP, isOutput=True)

    with tile.TileContext(nc) as tc, ExitStack() as ctx:
        consts = ctx.enter_context(tc.tile_pool(name="consts", bufs=1))
        persist = ctx.enter_context(tc.tile_pool(name="persist", bufs=1))

        # ---- transposed x via DMA-transpose (no PE transposes needed) ----
        # xkvT[c, ck, n] = xkv[n, 128*ck + c]; loaded in 512-row chunks so
        # the K/V projections can start before the whole tensor lands.
        xkvT = persist.tile([128, 2, N], BF)
        xqT = persist.tile([128, 2, NQ], BF)

        def ld_w(sb, src_t):
            for ck in range(2):
                nc.sync.dma_start(out=sb[:, ck, :],
                                  in_=src_t[128 * ck:128 * (ck + 1), :])

        def ld_kv(c):
            nc.sync.dma_start_transpose(out=xkvT[:, :, 512 * c:512 * (c + 1)],
                                        in_=xkv[512 * c:512 * (c + 1), :])

        def ld_q(c):
            nc.sync.dma_start_transpose(out=xqT[:, :, 512 * c:512 * (c + 1)],
                                        in_=xq[512 * c:512 * (c + 1), :])

        # ---- weights / consts; DMA order matters: the sync queue issues
        # serially, so wk/wq go first (kproj/qproj gate the loop start),
        # transposing x loads interleave, wout/bias trail ----
        wq_sb = consts.tile([128, 2, DIM], BF)
        wk_sb = consts.tile([128, 2, DIM], BF)
        wv_sb = consts.tile([128, 2, DIM], BF)
        wout_sb = consts.tile([128, 2, DIM], BF)
        ld_kv(0); ld_kv(1); ld_kv(2); ld_kv(3)
        ld_q(0); ld_q(1)
        ld_w(wk_sb, wk)
        ld_w(wv_sb, wv)
        ld_w(wq_sb, wq)
        ld_w(wout_sb, wout)
        bias_b = consts.tile([128, DIM], FP)
        nc.sync.dma_start(out=bias_b, in_=bout[:].unsqueeze(0).to_broadcast([128, DIM]))

        # ---- persistent attention state ----
        # qT/kT stacked-head layout: tensor i holds heads 4i..4i+3; row
        # 32*hloc + d <-> head 4i+hloc, dim d.
        qT = [persist.tile([128, NQ], BF, tag=f"qT{i}", name=f"qT{i}") for i in range(2)]
        kT = [persist.tile([128, N], BF, tag=f"kT{i}", name=f"kT{i}") for i in range(2)]
        # v in bf16, normal layout + ones column [128(n), t, h, 33]; the
        # ones column is exact in bf16 and yields the softmax denominator
        # from the same matmul that accumulates the numerator
        v_sb = persist.tile([128, NT_KV, NUM_HEADS, HEAD_DIM + 1], BF)
        nc.vector.memset(v_sb[:, :, :, HEAD_DIM:], 1.0)
        PTraw = [persist.tile([128, NQ], FP, tag=f"PTr{i}", name=f"PTr{i}") for i in range(2)]
        PTb = [persist.tile([128, NQ], BF, tag=f"PTb{i}", name=f"PTb{i}") for i in range(2)]
        rb = [persist.tile([128, NQ], FP, tag=f"rb{i}", name=f"rb{i}") for i in range(2)]
        # (g,hh) group gi=2g+hh lives at partition 32*gi (+hloc) so DVE ops
        # on a group's 4 rows start at a 32-aligned partition base
        denom = persist.tile([128, 512], FP)
        recip = persist.tile([128, 512], FP)

        with (
            tc.tile_pool(name="spsum", bufs=3, space="PSUM") as spsum,
            tc.tile_pool(name="npsum", bufs=1, space="PSUM") as npsum,
            tc.tile_pool(name="esb", bufs=4) as esb,
            tc.tile_pool(name="evac", bufs=4) as evac,
            tc.tile_pool(name="osb", bufs=4) as osb,
            tc.tile_pool(name="dscratch", bufs=1, space="DRAM") as dsc,
        ):
            recip_dram = dsc.tile([16, 512], FP)

            def emit_qproj(i, g):
                ps = spsum.tile([128, 1024], FP, tag="sp", name="pjq")
                for ck in range(2):
                    nc.tensor.matmul(
                        ps[:, 0:512], lhsT=wq_sb[:, ck, 128 * i:128 * (i + 1)],
                        rhs=xqT[:, ck, 512 * g:512 * (g + 1)],
                        start=(ck == 0), stop=(ck == 1))
                nc.vector.tensor_copy(qT[i][:, 512 * g:512 * (g + 1)], ps[:, 0:512])

            def emit_kproj(i, gs=None):
                gl = list(range(NGK) if gs is None else gs)
                for g0 in gl[::2]:
                    ps = spsum.tile([128, 1024], FP, tag="sp", name="pjk")
                    for o, g in enumerate((g0, g0 + 1)):
                        for ck in range(2):
                            nc.tensor.matmul(
                                ps[:, 512 * o:512 * (o + 1)],
                                lhsT=wk_sb[:, ck, 128 * i:128 * (i + 1)],
                                rhs=xkvT[:, ck, 512 * g:512 * (g + 1)],
                                start=(ck == 0), stop=(ck == 1))
                    for o, g in enumerate((g0, g0 + 1)):
                        nc.vector.tensor_copy(kT[i][:, 512 * g:512 * (g + 1)],
                                              ps[:, 512 * o:512 * (o + 1)])

            def emit_vproj(ts):
                for t in ts:
                    ps = spsum.tile([128, 1024], FP, tag="sp", name="pjv")
                    for ck in range(2):
                        nc.tensor.matmul(
                            ps[:, 0:DIM], lhsT=xkvT[:, ck, 128 * t:128 * (t + 1)],
                            rhs=wv_sb[:, ck, :],
                            start=(ck == 0), stop=(ck == 1))
                    nc.vector.tensor_copy(v_sb[:, t, :, 0:HEAD_DIM],
                                          ps[:, 0:DIM])

            def emit_block(g, hh, pp, hooks=None):
                nps = npsum.tile([HEAD_DIM + 1, 1024], FP, tag="np", name="np")
                for j in range(NJ):
                    if hooks and j in hooks:
                        for fn in hooks[j]:
                            fn()
                    sp = spsum.tile([128, 1024], FP, tag="sp", name="sp")
                    for uu in range(2):
                        hloc = 2 * pp + uu
                        r = 32 * hloc
                        nc.tensor.matmul(
                            sp[:, 512 * uu:512 * (uu + 1)],
                            lhsT=kT[hh][r:r + 32, 128 * j:128 * (j + 1)],
                            rhs=qT[hh][r:r + 32, 512 * g:512 * (g + 1)],
                            start=True, stop=True,
                            tile_position=(r, 0))
                    e = esb.tile([128, 1024], BF, tag="e", name="e")
                    nc.scalar.activation(e, sp, EXP)
                    for uu in range(2):
                        h = 4 * hh + 2 * pp + uu
                        nc.tensor.matmul(
                            nps[:, 512 * uu:512 * (uu + 1)],
                            lhsT=v_sb[:, j, h, :],
                            rhs=e[:, 512 * uu:512 * (uu + 1)],
                            start=(j == 0), stop=(j == NJ - 1))
                tmp = evac.tile([HEAD_DIM + 1, 1024], FP, tag="ev", name="ev")
                nc.vector.tensor_copy(tmp, nps)
                for uu in range(2):
                    hloc = 2 * pp + uu
                    nc.sync.dma_start(
                        out=PTraw[hh][32 * hloc:32 * hloc + 32,
                                      512 * g:512 * (g + 1)],
                        in_=tmp[0:HEAD_DIM, 512 * uu:512 * (uu + 1)])
                r = 32 * (2 * g + hh) + 2 * pp
                nc.sync.dma_start(out=denom[r:r + 2, :],
                                  in_=tmp[HEAD_DIM:HEAD_DIM + 1, :])

            def emit_norm(g, hh):
                # denom -> recip -> DRAM-bounce partition-broadcast ->
                # normalized bf16 PT for this (g, hh); runs off the PE/ACT
                r0 = 32 * (2 * g + hh)
                d0 = 4 * (2 * g + hh)
                # eps (1e-6) skipped: denom' = e^-C * sum(e^s) is ~1e2 here, so
                # the reference's +1e-6 changes nothing at fp32 resolution
                nc.vector.reciprocal(recip[r0:r0 + 4, :], denom[r0:r0 + 4, :])
                nc.sync.dma_start(out=recip_dram[d0:d0 + 4, :],
                                  in_=recip[r0:r0 + 4, :])
                for hloc in range(4):
                    nc.sync.dma_start(
                        out=rb[hh][32 * hloc:32 * hloc + 32,
                                   512 * g:512 * (g + 1)],
                        in_=recip_dram[d0 + hloc:d0 + hloc + 1, :].to_broadcast([32, 512]))
                nc.vector.tensor_mul(PTb[hh][:, 512 * g:512 * (g + 1)],
                                     PTraw[hh][:, 512 * g:512 * (g + 1)],
                                     rb[hh][:, 512 * g:512 * (g + 1)])

            def emit_outproj(ts):
                # both halves per row-tile; pairs of tiles share one psum tile
                tl = list(ts)
                for t0 in tl[::2]:
                    ps = spsum.tile([128, 1024], FP, tag="sp", name="spo")
                    for o, t in enumerate((t0, t0 + 1)):
                        for i in range(2):
                            nc.tensor.matmul(
                                ps[:, 512 * o:512 * o + DIM],
                                lhsT=PTb[i][:, 128 * t:128 * (t + 1)],
                                rhs=wout_sb[:, i, :],
                                start=(i == 0), stop=(i == 1))
                    for o, t in enumerate((t0, t0 + 1)):
                        ob = osb.tile([128, DIM], FP, tag="ob", name=f"ob{o}")
                        nc.vector.tensor_add(ob, ps[:, 512 * o:512 * o + DIM], bias_b)
                        nc.sync.dma_start(out=out[128 * t:128 * (t + 1), :], in_=ob)

            # ---- minimal pre-loop projections ----
            emit_kproj(0)
            emit_qproj(0, 0)
            emit_vproj(range(NT_KV))

            # ---- attention blocks; leftover projections, normalize
            # chains, and out-projection halves sit between blocks where
            # their inputs are long since ready ----
            after = {
                (0, 0, 0): [lambda: emit_qproj(0, 1)],
                (0, 0, 1): [lambda: emit_kproj(1),
                            lambda: emit_norm(0, 0)],
                (0, 1, 0): [lambda: emit_qproj(1, 0)],
                (0, 1, 1): [lambda: emit_qproj(1, 1),
                            lambda: emit_norm(1, 0)],
                (1, 0, 0): [],
                (1, 0, 1): [lambda: emit_norm(0, 1)],
                (1, 1, 0): [lambda: emit_outproj(range(0, 4))],
                (1, 1, 1): [lambda: emit_norm(1, 1),
                            lambda: emit_outproj(range(4, NT_Q))],
            }
            for hh in range(2):
                for g in range(NGQ):
                    for pp in range(2):
                        emit_block(g, hh, pp)
                        for fn in after[(hh, g, pp)]:
                            fn()

    if not nc.is_finalized():
        nc.finalize()
    return nc


_NC_CACHE = None


def _get_program():
    global _NC_CACHE
    if _NC_CACHE is None:
        _NC_CACHE = build_program()
    return _NC_CACHE


def kernel(x, Wqkv, Wout, bout, _trace=False, _trace_kwargs=None):
    x = np.asarray(x, dtype=np.float32)
    Wqkv = np.asarray(Wqkv, dtype=np.float32)
    Wout = np.asarray(Wout, dtype=np.float32)
    bout = np.asarray(bout, dtype=np.float32)

    bf = ml_dtypes.bfloat16
    scale = HEAD_DIM ** -0.5
    wq = np.ascontiguousarray((Wqkv[:, 0:DIM] * scale).astype(bf))
    wk = np.ascontiguousarray(Wqkv[:, DIM:2 * DIM].astype(bf))
    wv = np.ascontiguousarray(Wqkv[:, 2 * DIM:3 * DIM].astype(bf))
    wout_bf = np.ascontiguousarray(Wout.astype(bf))
    x_bf = x.astype(bf)

    in_maps = []
    for c in range(NCORES):
        bi, u = c // 2, c % 2
        in_maps.append({
            "xq": np.ascontiguousarray(x_bf[bi, u * NQ:(u + 1) * NQ, :]),
            "xkv": np.ascontiguousarray(x_bf[bi]),
            "wq": wq, "wk": wk, "wv": wv,
            "wout": wout_bf,
            "bout": bout,
        })

    nc = _get_program()
    kwargs = {}
    if _trace:
        kwargs["trace"] = True
        if _trace_kwargs:
            kwargs.update(_trace_kwargs)
    res = run_bass_kernel_spmd(nc, in_maps, core_ids=list(range(NCORES)), **kwargs)

    outf = np.empty((B, N, DIM), dtype=np.float32)
    for c in range(NCORES):
        bi, u = c // 2, c % 2
        outf[bi, u * NQ:(u + 1) * NQ, :] = res.results[c]["out"]
    if _trace:
        return outf, res
    return outf


# revision 33
# speedup vs baseline: 1.1592x; 1.1592x over previous
"""KeOps-style multi-head attention (unnormalized-exp softmax) on 8 trn2 cores.

Sharding: core c handles batch bi = c//2 and query rows u*1024..(u+1)*1024
(u = c%2), ALL 8 heads. Output is a pure concat over cores (no reduction).

Engine budget per core (under the sustained-load PE clock throttle to
~1.2 GHz): the ACT engine's 128 exp instructions ([128,1024] psum->sbuf,
~1.0-1.2us each) form the ~140us spine; everything else hides under it.
All matmuls are bf16 (1 col/cycle on the PE, FWL-eligible weight loads;
fp32 matmuls are 4x slower and poison FWL for the following matmul), with
fp32 PSUM accumulation. The host pre-casts x and weights to bf16, which
also halves input DMA bytes.

Structure per (g=512 queries, hh=head-tensor, pp=head-pair) block:
scores^T chunks [128 keys, 2x512 queries] via row-tiled K=32 bf16
matmuls (the two heads' matmuls overlap on different PE row strips) ->
exp on ACT -> numer^T accumulation matmuls with e^T as the moving
operand and [v | ones] as the stationary (the ones column yields the
softmax denominator for free).

Program order matters because engines execute in-order: the attention
blocks are emitted as soon as kT[0]/v/qT[0] exist; the remaining
projections, the per-(g,hh) denom->recip->broadcast->normalize chains,
and the output projection are placed between later blocks so they
overlap the ACT-bound loop. x arrives pre-transposed via DMA-transpose
loads (no PE transposes). PSUM: 3x2 banks scores + 2 banks numer.
"""

import numpy as np
import ml_dtypes
from contextlib import ExitStack

import concourse.bass as bass
import concourse.mybir as mybir
import concourse.tile as tile
from concourse import bacc
from concourse.bass_utils import run_bass_kernel_spmd

DIM = 256
NUM_HEADS = 8
HEAD_DIM = 32
B = 4
N = 2048
NQ = 1024          # query rows per core
NCORES = 8
FP = mybir.dt.float32
BF = mybir.dt.bfloat16
EXP = mybir.ActivationFunctionType.Exp

NT_KV = N // 128   # 16 n-tiles of kv rows
NGQ = NQ // 512    # 2 groups of 512 query cols in q^T
NGK = N // 512     # 4 groups in k^T free dim
NJ = N // 128      # 16 key chunks of 128
NT_Q = NQ // 128   # 8 output row tiles


def build_program():
    nc = bacc.Bacc()

    xq = nc.declare_dram_parameter("xq", [NQ, DIM], BF, isOutput=False)
    xkv = nc.declare_dram_parameter("xkv", [N, DIM], BF, isOutput=False)
    wq = nc.declare_dram_parameter("wq", [DIM, DIM], BF, isOutput=False)
    wk = nc.declare_dram_parameter("wk", [DIM, DIM], BF, isOutput=False)
    wv = nc.declare_dram_parameter("wv", [DIM, DIM], BF, isOutput=False)
    wout = nc.declare_dram_parameter("wout", [DIM, DIM], BF, isOutput=False)
    bout = nc.declare_dram_parameter("bout", [DIM], FP, isOutput=False)
    out = nc.declare_dram_parameter("out", [NQ, DIM], FP, isOutput=True)

    with tile.TileContext(nc) as tc, ExitStack() as ctx:
        consts = ctx.enter_context(tc.tile_pool(name="consts", bufs=1))
        persist = ctx.enter_context(tc.tile_pool(name="persist", bufs=1))

        # ---- transposed x via DMA-transpose (no PE transposes needed) ----
        # xkvT[c, ck, n] = xkv[n, 128*ck + c]; loaded in 512-row chunks so
        # the K/V projections can start before the whole tensor lands.
        xkvT = persist.tile([128, 2, N], BF)
        xqT = persist.tile([128, 2, NQ], BF)

        def ld_w(sb, src_t):
            for ck in range(2):
                nc.sync.dma_start(out=sb[:, ck, :],
                                  in_=src_t[128 * ck:128 * (ck + 1), :])

        def ld_kv(c):
            nc.sync.dma_start_transpose(out=xkvT[:, :, 512 * c:512 * (c + 1)],
                                        in_=xkv[512 * c:512 * (c + 1), :])

        def ld_q(c):
            nc.sync.dma_start_transpose(out=xqT[:, :, 512 * c:512 * (c + 1)],
                                        in_=xq[512 * c:512 * (c + 1), :])

        # ---- weights / consts; DMA order matters: the sync queue issues
        # serially, so wk/wq go first (kproj/qproj gate the loop start),
        # transposing x loads interleave, wout/bias trail ----
        wq_sb = consts.tile([128, 2, DIM], BF)
        wk_sb = consts.tile([128, 2, DIM], BF)
        wv_sb = consts.tile([128, 2, DIM], BF)
        wout_sb = consts.tile([128, 2, DIM], BF)
        ld_kv(0); ld_kv(1); ld_kv(2); ld_kv(3)
        ld_q(0); ld_q(1)
        ld_w(wk_sb, wk)
        ld_w(wv_sb, wv)
        ld_w(wq_sb, wq)
        ld_w(wout_sb, wout)
        bias_b = consts.tile([128, DIM], FP)
        nc.sync.dma_start(out=bias_b, in_=bout[:].unsqueeze(0).to_broadcast([128, DIM]))

        # ---- persistent attention state ----
        # qT/kT stacked-head layout: tensor i holds heads 4i..4i+3; row
        # 32*hloc + d <-> head 4i+hloc, dim d.
        qT = [persist.tile([128, NQ], BF, tag=f"qT{i}", name=f"qT{i}") for i in range(2)]
        kT = [persist.tile([128, N], BF, tag=f"kT{i}", name=f"kT{i}") for i in range(2)]
        # v in bf16, normal layout + ones column [128(n), t, h, 33]; the
        # ones column is exact in bf16 and yields the softmax denominator
        # from the same matmul that accumulates the numerator
        v_sb = persist.tile([128, NT_KV, NUM_HEADS, HEAD_DIM + 1], BF)
        nc.vector.memset(v_sb[:, :, :, HEAD_DIM:], 1.0)
        PTraw = [persist.tile([128, NQ], FP, tag=f"PTr{i}", name=f"PTr{i}") for i in range(2)]
        PTb = [persist.tile([128, NQ], BF, tag=f"PTb{i}", name=f"PTb{i}") for i in range(2)]
        rb = [persist.tile([128, NQ], FP, tag=f"rb{i}", name=f"rb{i}") for i in range(2)]
        # (g,hh) group gi=2g+hh lives at partition 32*gi (+hloc) so DVE ops
        # on a group's 4 rows start at a 32-aligned partition base
        denom = persist.tile([128, 512], FP)
        recip = persist.tile([128, 512], FP)

        with (
            tc.tile_pool(name="spsum", bufs=3, space="PSUM") as spsum,
            tc.tile_pool(name="npsum", bufs=1, space="PSUM") as npsum,
            tc.tile_pool(name="esb", bufs=4) as esb,
            tc.tile_pool(name="evac", bufs=4) as evac,
            tc.tile_pool(name="osb", bufs=4) as osb,
            tc.tile_pool(name="dscratch", bufs=1, space="DRAM") as dsc,
        ):
            recip_dram = dsc.tile([16, 512], FP)

            def emit_qproj(i, g):
                ps = spsum.tile([128, 1024], FP, tag="sp", name="pjq")
                for ck in range(2):
                    nc.tensor.matmul(
                        ps[:, 0:512], lhsT=wq_sb[:, ck, 128 * i:128 * (i + 1)],
                        rhs=xqT[:, ck, 512 * g:512 * (g + 1)],
                        start=(ck == 0), stop=(ck == 1))
                nc.vector.tensor_copy(qT[i][:, 512 * g:512 * (g + 1)], ps[:, 0:512])

            def emit_kproj(i, gs=None):
                gl = list(range(NGK) if gs is None else gs)
                for g0 in gl[::2]:
                    ps = spsum.tile([128, 1024], FP, tag="sp", name="pjk")
                    for o, g in enumerate((g0, g0 + 1)):
                        for ck in range(2):
                            nc.tensor.matmul(
                                ps[:, 512 * o:512 * (o + 1)],
                                lhsT=wk_sb[:, ck, 128 * i:128 * (i + 1)],
                                rhs=xkvT[:, ck, 512 * g:512 * (g + 1)],
                                start=(ck == 0), stop=(ck == 1))
                    for o, g in enumerate((g0, g0 + 1)):
                        nc.vector.tensor_copy(kT[i][:, 512 * g:512 * (g + 1)],
                                              ps[:, 512 * o:512 * (o + 1)])

            def emit_vproj(ts):
                for t in ts:
                    ps = spsum.tile([128, 1024], FP, tag="sp", name="pjv")
                    for ck in range(2):
                        nc.tensor.matmul(
                            ps[:, 0:DIM], lhsT=xkvT[:, ck, 128 * t:128 * (t + 1)],
                            rhs=wv_sb[:, ck, :],
                            start=(ck == 0), stop=(ck == 1))
                    nc.vector.tensor_copy(v_sb[:, t, :, 0:HEAD_DIM],
                                          ps[:, 0:DIM])

            def emit_block(g, hh, pp, hooks=None):
                nps = npsum.tile([HEAD_DIM + 1, 1024], FP, tag="np", name="np")
                for j in range(NJ):
                    if hooks and j in hooks:
                        for fn in hooks[j]:
                            fn()
                    sp = spsum.tile([128, 1024], FP, tag="sp", name="sp")
                    for uu in range(2):
                        hloc = 2 * pp + uu
                        r = 32 * hloc
                        nc.tensor.matmul(
                            sp[:, 512 * uu:512 * (uu + 1)],
                            lhsT=kT[hh][r:r + 32, 128 * j:128 * (j + 1)],
                            rhs=qT[hh][r:r + 32, 512 * g:512 * (g + 1)],
                            start=True, stop=True,
                            tile_position=(r, 0))
                    e = esb.tile([128, 1024], BF, tag="e", name="e")
                    nc.scalar.activation(e, sp, EXP)
                    for uu in range(2):
                        h = 4 * hh + 2 * pp + uu
                        nc.tensor.matmul(
                            nps[:, 512 * uu:512 * (uu + 1)],
                            lhsT=v_sb[:, j, h, :],
                            rhs=e[:, 512 * uu:512 * (uu + 1)],
                            start=(j == 0), stop=(j == NJ - 1))
                tmp = evac.tile([HEAD_DIM + 1, 1024], FP, tag="ev", name="ev")
                nc.vector.tensor_copy(tmp, nps)
                for uu in range(2):
                    hloc = 2 * pp + uu
                    nc.sync.dma_start(
                        out=PTraw[hh][32 * hloc:32 * hloc + 32,
                                      512 * g:512 * (g + 1)],
                        in_=tmp[0:HEAD_DIM, 512 * uu:512 * (uu + 1)])
                r = 32 * (2 * g + hh) + 2 * pp
                nc.sync.dma_start(out=denom[r:r + 2, :],
                                  in_=tmp[HEAD_DIM:HEAD_DIM + 1, :])

            def emit_norm(g, hh):
                # denom -> recip -> DRAM-bounce partition-broadcast ->
                # normalized bf16 PT for this (g, hh); split into column
                # halves so the second reciprocal overlaps the first
                # half's DMA round-trip, and broadcast all 4 head rows in
                # one 3D-strided DMA
                r0 = 32 * (2 * g + hh)
                d0 = 4 * (2 * g + hh)
                # eps (1e-6) skipped: denom' ~1e2 here, so the reference's
                # +1e-6 changes nothing at fp32 resolution
                for c0 in (0, 256):
                    nc.vector.reciprocal(recip[r0:r0 + 4, c0:c0 + 256],
                                         denom[r0:r0 + 4, c0:c0 + 256])
                    nc.sync.dma_start(out=recip_dram[d0:d0 + 4, c0:c0 + 256],
                                      in_=recip[r0:r0 + 4, c0:c0 + 256])
                    nc.sync.dma_start(
                        out=rb[hh][:, 512 * g + c0:512 * g + c0 + 256],
                        in_=recip_dram[d0:d0 + 4, c0:c0 + 256]
                        .unsqueeze(1).to_broadcast([4, 32, 256]))
                for c0 in (0, 256):
                    nc.vector.tensor_mul(
                        PTb[hh][:, 512 * g + c0:512 * g + c0 + 256],
                        PTraw[hh][:, 512 * g + c0:512 * g + c0 + 256],
                        rb[hh][:, 512 * g + c0:512 * g + c0 + 256])

            def emit_outproj(ts):
                # both halves per row-tile; pairs of tiles share one psum tile
                tl = list(ts)
                for t0 in tl[::2]:
                    ps = spsum.tile([128, 1024], FP, tag="sp", name="spo")
                    for o, t in enumerate((t0, t0 + 1)):
                        for i in range(2):
                            nc.tensor.matmul(
                                ps[:, 512 * o:512 * o + DIM],
                                lhsT=PTb[i][:, 128 * t:128 * (t + 1)],
                                rhs=wout_sb[:, i, :],
                                start=(i == 0), stop=(i == 1))
                    for o, t in enumerate((t0, t0 + 1)):
                        ob = osb.tile([128, DIM], FP, tag="ob", name=f"ob{o}")
                        nc.vector.tensor_add(ob, ps[:, 512 * o:512 * o + DIM], bias_b)
                        nc.sync.dma_start(out=out[128 * t:128 * (t + 1), :], in_=ob)

            # ---- minimal pre-loop projections ----
            emit_kproj(0)
            emit_qproj(0, 0)
            emit_vproj(range(NT_KV))

            # ---- attention blocks; leftover projections, normalize
            # chains, and out-projection halves sit between blocks where
            # their inputs are long since ready ----
            after = {
                (0, 0, 0): [lambda: emit_qproj(0, 1)],
                (0, 0, 1): [lambda: emit_kproj(1),
                            lambda: emit_norm(0, 0)],
                (0, 1, 0): [lambda: emit_qproj(1, 0)],
                (0, 1, 1): [lambda: emit_qproj(1, 1),
                            lambda: emit_norm(1, 0)],
                (1, 0, 0): [],
                (1, 0, 1): [lambda: emit_norm(0, 1)],
                (1, 1, 0): [lambda: emit_outproj(range(0, 4))],
                (1, 1, 1): [lambda: emit_norm(1, 1),
                            lambda: emit_outproj(range(4, NT_Q))],
            }
            for hh in range(2):
                for g in range(NGQ):
                    for pp in range(2):
                        emit_block(g, hh, pp)
                        for fn in after[(hh, g, pp)]:
                            fn()

    if not nc.is_finalized():
        nc.finalize()
    return nc


_NC_CACHE = None


def _get_program():
    global _NC_CACHE
    if _NC_CACHE is None:
        _NC_CACHE = build_program()
    return _NC_CACHE


def kernel(x, Wqkv, Wout, bout, _trace=False, _trace_kwargs=None):
    x = np.asarray(x, dtype=np.float32)
    Wqkv = np.asarray(Wqkv, dtype=np.float32)
    Wout = np.asarray(Wout, dtype=np.float32)
    bout = np.asarray(bout, dtype=np.float32)

    bf = ml_dtypes.bfloat16
    scale = HEAD_DIM ** -0.5
    wq = np.ascontiguousarray((Wqkv[:, 0:DIM] * scale).astype(bf))
    wk = np.ascontiguousarray(Wqkv[:, DIM:2 * DIM].astype(bf))
    wv = np.ascontiguousarray(Wqkv[:, 2 * DIM:3 * DIM].astype(bf))
    wout_bf = np.ascontiguousarray(Wout.astype(bf))
    x_bf = x.astype(bf)

    in_maps = []
    for c in range(NCORES):
        bi, u = c // 2, c % 2
        in_maps.append({
            "xq": np.ascontiguousarray(x_bf[bi, u * NQ:(u + 1) * NQ, :]),
            "xkv": np.ascontiguousarray(x_bf[bi]),
            "wq": wq, "wk": wk, "wv": wv,
            "wout": wout_bf,
            "bout": bout,
        })

    nc = _get_program()
    kwargs = {}
    if _trace:
        kwargs["trace"] = True
        if _trace_kwargs:
            kwargs.update(_trace_kwargs)
    res = run_bass_kernel_spmd(nc, in_maps, core_ids=list(range(NCORES)), **kwargs)

    outf = np.empty((B, N, DIM), dtype=np.float32)
    for c in range(NCORES):
        bi, u = c // 2, c % 2
        outf[bi, u * NQ:(u + 1) * NQ, :] = res.results[c]["out"]
    if _trace:
        return outf, res
    return outf
